# revision 4
# baseline (speedup 1.0000x reference)
"""Trainium2 Bass kernel for nn_MultiHeadAttention_60559038873660.

Reference math (faithful to the source bug: attention is contracted with the
projected K, not V, so v/Wv are dead inputs):
    qp = q @ Wq.T ; kp = k @ Wk.T
    head split via reshape(b, l, 64, 16): head n takes strided columns {d*16+n}
    S = Qh @ Kh.T / 8 ; A = softmax(S, axis=m) ; X = A @ Kh ; out = X @ Wo.T

Strategy:
  - Host-side: permute weight rows/cols head-major so each head is a contiguous
    64-column block; pre-transpose q/k/weights into the layouts the TensorE
    wants (contraction on partitions).
  - 8 cores = 2 batches x 4 head-groups (4 heads each).  Each core computes its
    4 heads' attention plus a partial output projection; the host sums the 4
    partials per batch (tensor-parallel row-split reduction).
  - On-core dataflow (all matmuls float32r = full-rate fp32, rel err ~1e-4):
      QhT[c,l], KhT[c,m]  : projections with contraction over DIM
      Kh[m,c(+ones)]      : second projection of k, with a ones column fused so
                            the attention row-sums (softmax denominators) fall
                            out of the X^T matmul for free
      S^T[m,l] = KhT.T@QhT per head ; exp on ScalarE (scale=1/8) PSUM->SBUF
      X^T[d+1,l] accumulated over m-chunks; row 64 = denominators
      normalize via reciprocal + DRAM-broadcast + VectorE multiply
      out_partial[l,j] = Xn^T.T @ WoT
"""

import contextlib
import ctypes
import os
import sys
import types

import numpy as np

import concourse.bacc as bacc
import concourse.tile as tile
from concourse import mybir
from concourse.bass import ds, ts
from concourse.bass_utils import run_bass_kernel_spmd


def _install_ntff_hook():
    """Provide antenv.axon_hooks if the image lacks it, wiring NTFF
    profiling straight into libaxon_pjrt.so (same ABI trn_boot uses)."""
    try:
        import antenv.axon_hooks  # noqa: F401
        return
    except ImportError:
        pass
    mod = types.ModuleType("antenv.axon_hooks")
    holder = [None]
    mod.set_axon_ntff_profile_hook = lambda h: holder.__setitem__(0, h)
    mod.get_axon_ntff_profile_hook = lambda: holder[0]
    sys.modules["antenv.axon_hooks"] = mod
    try:
        import antenv
        antenv.axon_hooks = mod
    except ImportError:
        pass

    so_path = "/opt/axon/libaxon_pjrt.so"
    if not os.path.exists(so_path):
        return
    lib = ctypes.CDLL(so_path)
    if not hasattr(lib, "axon_start_nrt_profile"):
        return
    lib.axon_start_nrt_profile.argtypes = [ctypes.POINTER(ctypes.c_int64), ctypes.c_size_t]
    lib.axon_start_nrt_profile.restype = ctypes.c_int64
    lib.axon_stop_nrt_profile.argtypes = [ctypes.c_char_p]
    lib.axon_stop_nrt_profile.restype = ctypes.c_int64

    @contextlib.contextmanager
    def _hook(output_dir, device_ids):
        import jax
        jax.devices()
        if device_ids:
            ids = (ctypes.c_int64 * len(device_ids))(*device_ids)
            rc = lib.axon_start_nrt_profile(ids, len(device_ids))
        else:
            rc = lib.axon_start_nrt_profile(None, 0)
        if rc != 0:
            raise RuntimeError(f"axon_start_nrt_profile rc={rc}")
        try:
            yield
        finally:
            n = lib.axon_stop_nrt_profile(str(output_dir).encode())
            print(f"profile: {n} file(s) written to {output_dir}", file=sys.stderr)

    mod.set_axon_ntff_profile_hook(_hook)


_install_ntff_hook()

f32 = mybir.dt.float32
f32r = mybir.dt.float32r
Exp = mybir.ActivationFunctionType.Exp

P = 128
DIM = 1024
NH = 16
HD = 64
HPC = 4          # heads per core
CW = HPC * HD    # 256 channel columns per core
CH = HD + 1      # head channels + ones column
G = CW // P      # 2 channel groups of 128
KC = DIM // P    # 8 contraction chunks for projections
JT = DIM // 512  # out-projection j tiles

_cache = {}


def _build(L, M):
    NT = min(512, L)          # matmul moving-dim tile
    LT = L // NT
    MT = M // NT
    SUB = NT // P
    MG = M // P               # m chunks for attention
    LSTRIP = min(1024, L)     # attention l-strip (PSUM budget)
    LS = L // LSTRIP
    LN = LSTRIP // NT

    nc = bacc.Bacc()
    qT = nc.declare_dram_parameter("qT", [DIM, L], f32, isOutput=False)
    kT = nc.declare_dram_parameter("kT", [DIM, M], f32, isOutput=False)
    wqT = nc.declare_dram_parameter("wqT", [DIM, CW], f32, isOutput=False)
    wkT = nc.declare_dram_parameter("wkT", [DIM, CW], f32, isOutput=False)
    woT = nc.declare_dram_parameter("woT", [CW, DIM], f32, isOutput=False)
    out = nc.declare_dram_parameter("out", [L, DIM], f32, isOutput=True)
    rden_dram = nc.dram_tensor("rden_scratch", [HPC, L], f32)

    with tile.TileContext(nc) as tc:
        with (
            tc.tile_pool(name="singles", bufs=1) as singles,
            tc.tile_pool(name="io", bufs=3) as io,
            tc.tile_pool(name="es", bufs=3) as es_pool,
            tc.tile_pool(name="opool", bufs=3) as opool,
        ):
            wq_sb = singles.tile([P, KC, CW], f32r)
            nc.sync.dma_start(wq_sb, wqT.rearrange("(kc p) c -> p kc c", p=P).bitcast(f32r))
            wk_sb = singles.tile([P, KC, CW], f32r)
            nc.sync.dma_start(wk_sb, wkT.rearrange("(kc p) c -> p kc c", p=P).bitcast(f32r))
            wo_sb = singles.tile([P, G, DIM], f32r)
            nc.sync.dma_start(wo_sb, woT.rearrange("(g p) j -> p g j", p=P).bitcast(f32r))

            qhT = singles.tile([P, G, L], f32r)
            khT = singles.tile([P, G, M], f32r)
            khp = singles.tile([P, MG, HPC, CH], f32r)
            xu = singles.tile([P, G, L], f32r)
            dstage = singles.tile([1, HPC, L], f32)
            rdbc = singles.tile([P, G, L], f32)

            ones_sb = singles.tile([P, 1], f32)
            nc.vector.memset(ones_sb, 1.0)
            for mg in range(MG):
                nc.vector.tensor_copy(khp[:, mg, :, HD:CH],
                                      ones_sb[:, None, :].to_broadcast([P, HPC, 1]))

            # ---- projections ----
            with tc.tile_pool(name="psP", bufs=2, space="PSUM") as psP:
                # QhT[c, l] = wqT.T @ qT
                for lt in range(LT):
                    qt_t = io.tile([P, KC, NT], f32r, tag="io")
                    nc.sync.dma_start(
                        qt_t, qT[:, ts(lt, NT)].rearrange("(kc p) l -> p kc l", p=P).bitcast(f32r))
                    for g in range(G):
                        ps = psP.tile([P, NT], f32, tag="ps")
                        for kc in range(KC):
                            nc.tensor.matmul(ps, lhsT=wq_sb[:, kc, ts(g, P)], rhs=qt_t[:, kc],
                                             start=(kc == 0), stop=(kc == KC - 1))
                        nc.vector.tensor_copy(qhT[:, g, ts(lt, NT)], ps)
                # KhT[c, m] and Kh[m, c] from the same k chunks
                for mt in range(MT):
                    kt_t = io.tile([P, KC, NT], f32r, tag="io")
                    nc.sync.dma_start(
                        kt_t, kT[:, ts(mt, NT)].rearrange("(kc p) m -> p kc m", p=P).bitcast(f32r))
                    for g in range(G):
                        ps = psP.tile([P, NT], f32, tag="ps")
                        for kc in range(KC):
                            nc.tensor.matmul(ps, lhsT=wk_sb[:, kc, ts(g, P)], rhs=kt_t[:, kc],
                                             start=(kc == 0), stop=(kc == KC - 1))
                        nc.vector.tensor_copy(khT[:, g, ts(mt, NT)], ps)
                    for sub in range(SUB):
                        mg = mt * SUB + sub
                        ps2 = psP.tile([P, CW], f32, tag="ps2")
                        for kc in range(KC):
                            nc.tensor.matmul(ps2, lhsT=kt_t[:, kc, ts(sub, P)], rhs=wk_sb[:, kc, :],
                                             start=(kc == 0), stop=(kc == KC - 1))
                        for h in range(HPC):
                            nc.vector.tensor_copy(khp[:, mg, h, 0:HD], ps2[:, ts(h, HD)])

            # ---- attention ----
            with (
                tc.tile_pool(name="psS", bufs=2, space="PSUM") as psS,
                tc.tile_pool(name="psX", bufs=2, space="PSUM") as psX,
            ):
                for h in range(HPC):
                    g, hh = divmod(h, 2)
                    pb = hh * HD
                    for lsi in range(LS):
                        xps = psX.tile([CH, LSTRIP], f32, tag="x")
                        for mc in range(MG):
                            sps = psS.tile([P, LSTRIP], f32, tag="s")
                            for ln in range(LN):
                                nc.tensor.matmul(
                                    sps[:, ts(ln, NT)],
                                    lhsT=khT[pb:pb + HD, g, ts(mc, P)],
                                    rhs=qhT[pb:pb + HD, g, ds(lsi * LSTRIP + ln * NT, NT)],
                                    start=True, stop=True)
                            es = es_pool.tile([P, LSTRIP], f32r, tag="es")
                            nc.scalar.activation(es, sps, Exp, scale=0.125)
                            for ln in range(LN):
                                nc.tensor.matmul(
                                    xps[:, ts(ln, NT)],
                                    lhsT=khp[:, mc, h, :],
                                    rhs=es[:, ts(ln, NT)],
                                    start=(mc == 0), stop=(mc == MG - 1))
                        nc.vector.tensor_copy(xu[pb:pb + HD, g, ds(lsi * LSTRIP, LSTRIP)], xps[0:HD])
                        nc.vector.tensor_copy(dstage[0:1, h, ds(lsi * LSTRIP, LSTRIP)], xps[HD:CH])

            # ---- normalize ----
            nc.vector.reciprocal(dstage, dstage)
            nc.sync.dma_start(rden_dram[:, :].unsqueeze(0), dstage[0:1, :, :])
            for h in range(HPC):
                g, hh = divmod(h, 2)
                nc.sync.dma_start(rdbc[ts(hh, HD), g, :], rden_dram[h:h + 1, :].to_broadcast([HD, L]))
            for g in range(G):
                nc.vector.tensor_mul(xu[:, g, :], xu[:, g, :], rdbc[:, g, :])

            # ---- output projection ----
            with tc.tile_pool(name="psO", bufs=4, space="PSUM") as psO:
                for lc in range(L // P):
                    for jt in range(JT):
                        po = psO.tile([P, 512], f32, tag="po")
                        for cc in range(G):
                            nc.tensor.matmul(po, lhsT=xu[:, cc, ts(lc, P)],
                                             rhs=wo_sb[:, cc, ts(jt, 512)],
                                             start=(cc == 0), stop=(cc == G - 1))
                        ot = opool.tile([P, 512], f32, tag="ot")
                        nc.vector.tensor_copy(ot, po)
                        nc.sync.dma_start(out[ts(lc, P), ts(jt, 512)], ot)

    nc.finalize()
    return nc


def _get_nc(L, M):
    key = (L, M)
    if key not in _cache:
        _cache[key] = _build(L, M)
    return _cache[key]


# head-major channel permutation: new channel c = h*64+d <- original column d*16+h
_PERM = np.array([(c % HD) * NH + c // HD for c in range(DIM)])

last_exec_time_ns = None
last_results = None


def kernel(q, k, v, Wq, Wk, Wv, Wo):  # noqa: ARG001 - v/Wv dead in reference
    global last_exec_time_ns, last_results
    q = np.asarray(q, np.float32)
    k = np.asarray(k, np.float32)
    Wq = np.asarray(Wq, np.float32)
    Wk = np.asarray(Wk, np.float32)
    Wo = np.asarray(Wo, np.float32)
    B, L, _ = q.shape
    M = k.shape[1]

    Wq_p = Wq[_PERM]            # (1024, 1024) head-major rows
    Wk_p = Wk[_PERM]
    WoT_p = Wo[:, _PERM].T      # (1024 c, 1024 j)

    qT = [np.ascontiguousarray(q[b].T) for b in range(B)]
    kT = [np.ascontiguousarray(k[b].T) for b in range(B)]
    wqT = [np.ascontiguousarray(Wq_p[hg * CW:(hg + 1) * CW, :].T) for hg in range(4)]
    wkT = [np.ascontiguousarray(Wk_p[hg * CW:(hg + 1) * CW, :].T) for hg in range(4)]
    woT = [np.ascontiguousarray(WoT_p[hg * CW:(hg + 1) * CW, :]) for hg in range(4)]

    in_maps = []
    for core in range(8):
        b, hg = divmod(core, 4)
        in_maps.append({"qT": qT[b], "kT": kT[b], "wqT": wqT[hg],
                        "wkT": wkT[hg], "woT": woT[hg]})

    nc = _get_nc(L, M)
    trace = bool(int(os.environ.get("MHA_TRACE", "0")))
    res = run_bass_kernel_spmd(nc, in_maps, core_ids=list(range(8)), trace=trace)
    last_results = res
    last_exec_time_ns = res.exec_time_ns

    out = np.zeros((B, L, DIM), np.float32)
    for core in range(8):
        b = core // 4
        out[b] += res.results[core]["out"]
    return out


# revision 8
# speedup vs baseline: 1.1123x; 1.1123x over previous
"""Trainium2 Bass kernel for nn_MultiHeadAttention_60559038873660.

Reference math (faithful to the source bug: attention is contracted with the
projected K, not V, so v/Wv are dead inputs):
    qp = q @ Wq.T ; kp = k @ Wk.T
    head split via reshape(b, l, 64, 16): head n takes strided columns {d*16+n}
    S = Qh @ Kh.T / 8 ; A = softmax(S, axis=m) ; X = A @ Kh ; out = X @ Wo.T

Strategy:
  - Host-side: permute weight rows/cols head-major so each head is a contiguous
    64-column block; pre-transpose q/k/weights into the layouts the TensorE
    wants (contraction on partitions).
  - 8 cores = 2 batches x 4 head-groups (4 heads each).  Each core computes its
    4 heads' attention plus a partial output projection; the host sums the 4
    partials per batch (tensor-parallel row-split reduction).
  - On-core dataflow (all matmuls float32r = full-rate fp32, rel err ~1e-4):
      QhT[c,l], KhT[c,m]  : projections with contraction over DIM
      Kh[m,c(+ones)]      : second projection of k, with a ones column fused so
                            the attention row-sums (softmax denominators) fall
                            out of the X^T matmul for free
      S^T[m,l] = KhT.T@QhT per head ; exp on ScalarE (scale=1/8) PSUM->SBUF
      X^T[d+1,l] accumulated over m-chunks; row 64 = denominators
      normalize via reciprocal + DRAM-broadcast + VectorE multiply
      out_partial[l,j] = Xn^T.T @ WoT
"""

import contextlib
import ctypes
import os
import sys
import types

import numpy as np

import concourse.bacc as bacc
import concourse.tile as tile
from concourse import mybir
from concourse.bass import ds, ts
from concourse.bass_utils import run_bass_kernel_spmd


def _install_ntff_hook():
    """Provide antenv.axon_hooks if the image lacks it, wiring NTFF
    profiling straight into libaxon_pjrt.so (same ABI trn_boot uses)."""
    try:
        import antenv.axon_hooks  # noqa: F401
        return
    except ImportError:
        pass
    mod = types.ModuleType("antenv.axon_hooks")
    holder = [None]
    mod.set_axon_ntff_profile_hook = lambda h: holder.__setitem__(0, h)
    mod.get_axon_ntff_profile_hook = lambda: holder[0]
    sys.modules["antenv.axon_hooks"] = mod
    try:
        import antenv
        antenv.axon_hooks = mod
    except ImportError:
        pass

    so_path = "/opt/axon/libaxon_pjrt.so"
    if not os.path.exists(so_path):
        return
    lib = ctypes.CDLL(so_path)
    if not hasattr(lib, "axon_start_nrt_profile"):
        return
    lib.axon_start_nrt_profile.argtypes = [ctypes.POINTER(ctypes.c_int64), ctypes.c_size_t]
    lib.axon_start_nrt_profile.restype = ctypes.c_int64
    lib.axon_stop_nrt_profile.argtypes = [ctypes.c_char_p]
    lib.axon_stop_nrt_profile.restype = ctypes.c_int64

    @contextlib.contextmanager
    def _hook(output_dir, device_ids):
        import jax
        jax.devices()
        if device_ids:
            ids = (ctypes.c_int64 * len(device_ids))(*device_ids)
            rc = lib.axon_start_nrt_profile(ids, len(device_ids))
        else:
            rc = lib.axon_start_nrt_profile(None, 0)
        if rc != 0:
            raise RuntimeError(f"axon_start_nrt_profile rc={rc}")
        try:
            yield
        finally:
            n = lib.axon_stop_nrt_profile(str(output_dir).encode())
            print(f"profile: {n} file(s) written to {output_dir}", file=sys.stderr)

    mod.set_axon_ntff_profile_hook(_hook)


_install_ntff_hook()

f32 = mybir.dt.float32
f32r = mybir.dt.float32r
Exp = mybir.ActivationFunctionType.Exp

P = 128
DIM = 1024
NH = 16
HD = 64
HPC = 4          # heads per core
CW = HPC * HD    # 256 channel columns per core
CH = HD + 1      # head channels + ones column
G = CW // P      # 2 channel groups of 128
KC = DIM // P    # 8 contraction chunks for projections
JT = DIM // 512  # out-projection j tiles

_cache = {}


def _build(L, M):
    NT = min(512, L)          # matmul moving-dim tile
    LT = L // NT
    MT = M // NT
    SUB = NT // P
    MG = M // P               # m chunks for attention
    LSTRIP = min(1024, L)     # attention l-strip (PSUM budget)
    LS = L // LSTRIP
    LN = LSTRIP // NT

    nc = bacc.Bacc()
    qT = nc.declare_dram_parameter("qT", [DIM, L], f32, isOutput=False)
    kT = nc.declare_dram_parameter("kT", [DIM, M], f32, isOutput=False)
    wqT = nc.declare_dram_parameter("wqT", [DIM, CW], f32, isOutput=False)
    wkT = nc.declare_dram_parameter("wkT", [DIM, CW], f32, isOutput=False)
    woT = nc.declare_dram_parameter("woT", [CW, DIM], f32, isOutput=False)
    out = nc.declare_dram_parameter("out", [L, DIM], f32, isOutput=True)
    den_dram = nc.dram_tensor("den_scratch", [HPC, L], f32)
    rden_dram = nc.dram_tensor("rden_scratch", [HPC, L], f32)

    with tile.TileContext(nc) as tc:
        with (
            tc.tile_pool(name="singles", bufs=1) as singles,
            tc.tile_pool(name="io", bufs=3) as io,
            tc.tile_pool(name="es", bufs=3) as es_pool,
            tc.tile_pool(name="opool", bufs=3) as opool,
        ):
            wq_sb = singles.tile([P, KC, CW], f32r)
            nc.sync.dma_start(wq_sb, wqT.rearrange("(kc p) c -> p kc c", p=P).bitcast(f32r))
            wk_sb = singles.tile([P, KC, CW], f32r)
            nc.sync.dma_start(wk_sb, wkT.rearrange("(kc p) c -> p kc c", p=P).bitcast(f32r))
            wo_sb = singles.tile([P, G, DIM], f32r)
            nc.sync.dma_start(wo_sb, woT.rearrange("(g p) j -> p g j", p=P).bitcast(f32r))

            qhT = singles.tile([P, G, L], f32r)
            khT = singles.tile([P, G, M], f32r)
            khp = singles.tile([P, MG, HPC, CH], f32r)
            xu = singles.tile([P, G, L], f32r)
            dstage = singles.tile([1, HPC, L], f32)
            rdbc = singles.tile([P, G, L], f32)

            ones_sb = singles.tile([P, 1], f32)
            nc.vector.memset(ones_sb, 1.0)
            for mg in range(MG):
                nc.vector.tensor_copy(khp[:, mg, :, HD:CH],
                                      ones_sb[:, None, :].to_broadcast([P, HPC, 1]))

            # ---- projections ----
            with tc.tile_pool(name="psP", bufs=2, space="PSUM") as psP:
                # QhT[c, l] = wqT.T @ qT
                for lt in range(LT):
                    qt_t = io.tile([P, KC, NT], f32r, tag="io")
                    nc.sync.dma_start(
                        qt_t, qT[:, ts(lt, NT)].rearrange("(kc p) l -> p kc l", p=P).bitcast(f32r))
                    for g in range(G):
                        ps = psP.tile([P, NT], f32, tag="ps")
                        for kc in range(KC):
                            nc.tensor.matmul(ps, lhsT=wq_sb[:, kc, ts(g, P)], rhs=qt_t[:, kc],
                                             start=(kc == 0), stop=(kc == KC - 1))
                        nc.vector.tensor_copy(qhT[:, g, ts(lt, NT)], ps)
                # KhT[c, m] and Kh[m, c] from the same k chunks
                for mt in range(MT):
                    kt_t = io.tile([P, KC, NT], f32r, tag="io")
                    nc.sync.dma_start(
                        kt_t, kT[:, ts(mt, NT)].rearrange("(kc p) m -> p kc m", p=P).bitcast(f32r))
                    for g in range(G):
                        ps = psP.tile([P, NT], f32, tag="ps")
                        for kc in range(KC):
                            nc.tensor.matmul(ps, lhsT=wk_sb[:, kc, ts(g, P)], rhs=kt_t[:, kc],
                                             start=(kc == 0), stop=(kc == KC - 1))
                        nc.vector.tensor_copy(khT[:, g, ts(mt, NT)], ps)
                    for sub in range(SUB):
                        mg = mt * SUB + sub
                        ps2 = psP.tile([P, CW], f32, tag="ps2")
                        for kc in range(KC):
                            nc.tensor.matmul(ps2, lhsT=kt_t[:, kc, ts(sub, P)], rhs=wk_sb[:, kc, :],
                                             start=(kc == 0), stop=(kc == KC - 1))
                        for h in range(HPC):
                            nc.vector.tensor_copy(khp[:, mg, h, 0:HD], ps2[:, ts(h, HD)])

            # ---- attention ----
            # Software-pipelined emission: S(mc+1) is enqueued on the PE
            # before X(mc) so the PE never head-of-line blocks on exp(mc)
            # (keeps TensorE dense -> HAM stays at full clock).
            with (
                tc.tile_pool(name="psS", bufs=3, space="PSUM") as psS,
                tc.tile_pool(name="psX", bufs=1, space="PSUM") as psX,
            ):
                for h in range(HPC):
                    g, hh = divmod(h, 2)
                    pb = hh * HD

                    for lsi in range(LS):
                        def emit_s(mc, lsi=lsi, g=g, pb=pb):
                            sps = psS.tile([P, LSTRIP], f32, tag="s")
                            for ln in range(LN):
                                nc.tensor.matmul(
                                    sps[:, ts(ln, NT)],
                                    lhsT=khT[pb:pb + HD, g, ts(mc, P)],
                                    rhs=qhT[pb:pb + HD, g, ds(lsi * LSTRIP + ln * NT, NT)],
                                    start=True, stop=True)
                            return sps

                        xps = psX.tile([CH, LSTRIP], f32, tag="x")
                        sps_cur = emit_s(0)
                        for mc in range(MG):
                            sps_next = emit_s(mc + 1) if mc + 1 < MG else None
                            es = es_pool.tile([P, LSTRIP], f32r, tag="es")
                            nc.scalar.activation(es, sps_cur, Exp, scale=0.125)
                            for ln in range(LN):
                                nc.tensor.matmul(
                                    xps[:, ts(ln, NT)],
                                    lhsT=khp[:, mc, h, :],
                                    rhs=es[:, ts(ln, NT)],
                                    start=(mc == 0), stop=(mc == MG - 1))
                            sps_cur = sps_next
                        nc.vector.tensor_copy(xu[pb:pb + HD, g, ds(lsi * LSTRIP, LSTRIP)], xps[0:HD])
                        nc.vector.tensor_copy(dstage[0:1, h, ds(lsi * LSTRIP, LSTRIP)], xps[HD:CH])

            # ---- normalize ----
            # reciprocal on one partition is ~50us; bounce through DRAM to
            # spread the 4*L denominators over 128 partitions first.
            FSP = HPC * L // P
            dsp = singles.tile([P, FSP], f32)
            nc.sync.dma_start(den_dram[:, :].unsqueeze(0), dstage[0:1, :, :])
            nc.sync.dma_start(dsp, den_dram[:, :].rearrange("h (p f) -> (h p) f", p=P // HPC))
            nc.vector.reciprocal(dsp, dsp)
            nc.sync.dma_start(rden_dram[:, :].rearrange("h (p f) -> (h p) f", p=P // HPC), dsp)
            for h in range(HPC):
                g, hh = divmod(h, 2)
                nc.sync.dma_start(rdbc[ts(hh, HD), g, :], rden_dram[h:h + 1, :].to_broadcast([HD, L]))
            for g in range(G):
                nc.vector.tensor_mul(xu[:, g, :], xu[:, g, :], rdbc[:, g, :])

            # ---- output projection ----
            with tc.tile_pool(name="psO", bufs=4, space="PSUM") as psO:
                for lc in range(L // P):
                    for jt in range(JT):
                        po = psO.tile([P, 512], f32, tag="po")
                        for cc in range(G):
                            nc.tensor.matmul(po, lhsT=xu[:, cc, ts(lc, P)],
                                             rhs=wo_sb[:, cc, ts(jt, 512)],
                                             start=(cc == 0), stop=(cc == G - 1))
                        ot = opool.tile([P, 512], f32, tag="ot")
                        nc.vector.tensor_copy(ot, po)
                        nc.sync.dma_start(out[ts(lc, P), ts(jt, 512)], ot)

    nc.finalize()
    return nc


def _get_nc(L, M):
    key = (L, M)
    if key not in _cache:
        _cache[key] = _build(L, M)
    return _cache[key]


# head-major channel permutation: new channel c = h*64+d <- original column d*16+h
_PERM = np.array([(c % HD) * NH + c // HD for c in range(DIM)])

last_exec_time_ns = None
last_results = None


def kernel(q, k, v, Wq, Wk, Wv, Wo):  # noqa: ARG001 - v/Wv dead in reference
    global last_exec_time_ns, last_results
    q = np.asarray(q, np.float32)
    k = np.asarray(k, np.float32)
    Wq = np.asarray(Wq, np.float32)
    Wk = np.asarray(Wk, np.float32)
    Wo = np.asarray(Wo, np.float32)
    B, L, _ = q.shape
    M = k.shape[1]

    Wq_p = Wq[_PERM]            # (1024, 1024) head-major rows
    Wk_p = Wk[_PERM]
    WoT_p = Wo[:, _PERM].T      # (1024 c, 1024 j)

    qT = [np.ascontiguousarray(q[b].T) for b in range(B)]
    kT = [np.ascontiguousarray(k[b].T) for b in range(B)]
    wqT = [np.ascontiguousarray(Wq_p[hg * CW:(hg + 1) * CW, :].T) for hg in range(4)]
    wkT = [np.ascontiguousarray(Wk_p[hg * CW:(hg + 1) * CW, :].T) for hg in range(4)]
    woT = [np.ascontiguousarray(WoT_p[hg * CW:(hg + 1) * CW, :]) for hg in range(4)]

    in_maps = []
    for core in range(8):
        b, hg = divmod(core, 4)
        in_maps.append({"qT": qT[b], "kT": kT[b], "wqT": wqT[hg],
                        "wkT": wkT[hg], "woT": woT[hg]})

    nc = _get_nc(L, M)
    trace = bool(int(os.environ.get("MHA_TRACE", "0")))
    res = run_bass_kernel_spmd(nc, in_maps, core_ids=list(range(8)), trace=trace)
    last_results = res
    last_exec_time_ns = res.exec_time_ns

    out = np.zeros((B, L, DIM), np.float32)
    for core in range(8):
        b = core // 4
        out[b] += res.results[core]["out"]
    return out


# revision 10
# speedup vs baseline: 1.6025x; 1.4408x over previous
"""Trainium2 Bass kernel for nn_MultiHeadAttention_60559038873660.

Reference math (faithful to the source bug: attention is contracted with the
projected K, not V, so v/Wv are dead inputs):
    qp = q @ Wq.T ; kp = k @ Wk.T
    head split via reshape(b, l, 64, 16): head n takes strided columns {d*16+n}
    S = Qh @ Kh.T / 8 ; A = softmax(S, axis=m) ; X = A @ Kh ; out = X @ Wo.T

Strategy:
  - Host-side: permute weight rows/cols head-major so each head is a contiguous
    64-column block; pre-transpose q/k/weights into the layouts the TensorE
    wants (contraction on partitions).
  - 8 cores = 2 batches x 4 head-groups (4 heads each).  Each core computes its
    4 heads' attention plus a partial output projection; the host sums the 4
    partials per batch (tensor-parallel row-split reduction).
  - On-core dataflow (all matmuls float32r = full-rate fp32, rel err ~1e-4):
      QhT[c,l], KhT[c,m]  : projections with contraction over DIM
      Kh[m,c(+ones)]      : second projection of k, with a ones column fused so
                            the attention row-sums (softmax denominators) fall
                            out of the X^T matmul for free
      S^T[m,l] = KhT.T@QhT per head ; exp on ScalarE (scale=1/8) PSUM->SBUF
      X^T[d+1,l] accumulated over m-chunks; row 64 = denominators
      normalize via reciprocal + DRAM-broadcast + VectorE multiply
      out_partial[l,j] = Xn^T.T @ WoT
"""

import contextlib
import ctypes
import os
import sys
import types

import numpy as np

import concourse.bacc as bacc
import concourse.tile as tile
from concourse import mybir
from concourse.bass import ds, ts
from concourse.bass_utils import run_bass_kernel_spmd


def _install_ntff_hook():
    """Provide antenv.axon_hooks if the image lacks it, wiring NTFF
    profiling straight into libaxon_pjrt.so (same ABI trn_boot uses)."""
    try:
        import antenv.axon_hooks  # noqa: F401
        return
    except ImportError:
        pass
    mod = types.ModuleType("antenv.axon_hooks")
    holder = [None]
    mod.set_axon_ntff_profile_hook = lambda h: holder.__setitem__(0, h)
    mod.get_axon_ntff_profile_hook = lambda: holder[0]
    sys.modules["antenv.axon_hooks"] = mod
    try:
        import antenv
        antenv.axon_hooks = mod
    except ImportError:
        pass

    so_path = "/opt/axon/libaxon_pjrt.so"
    if not os.path.exists(so_path):
        return
    lib = ctypes.CDLL(so_path)
    if not hasattr(lib, "axon_start_nrt_profile"):
        return
    lib.axon_start_nrt_profile.argtypes = [ctypes.POINTER(ctypes.c_int64), ctypes.c_size_t]
    lib.axon_start_nrt_profile.restype = ctypes.c_int64
    lib.axon_stop_nrt_profile.argtypes = [ctypes.c_char_p]
    lib.axon_stop_nrt_profile.restype = ctypes.c_int64

    @contextlib.contextmanager
    def _hook(output_dir, device_ids):
        import jax
        jax.devices()
        if device_ids:
            ids = (ctypes.c_int64 * len(device_ids))(*device_ids)
            rc = lib.axon_start_nrt_profile(ids, len(device_ids))
        else:
            rc = lib.axon_start_nrt_profile(None, 0)
        if rc != 0:
            raise RuntimeError(f"axon_start_nrt_profile rc={rc}")
        try:
            yield
        finally:
            n = lib.axon_stop_nrt_profile(str(output_dir).encode())
            print(f"profile: {n} file(s) written to {output_dir}", file=sys.stderr)

    mod.set_axon_ntff_profile_hook(_hook)


_install_ntff_hook()

f32 = mybir.dt.float32
f32r = mybir.dt.float32r
bf16 = mybir.dt.bfloat16
Exp = mybir.ActivationFunctionType.Exp

P = 128
DIM = 1024
NH = 16
HD = 64
HPC = 4          # heads per core
CW = HPC * HD    # 256 channel columns per core
CH = HD + 1      # head channels + ones column
G = CW // P      # 2 channel groups of 128
KC = DIM // P    # 8 contraction chunks for projections
JT = DIM // 512  # out-projection j tiles

_cache = {}


def _build(L, M):
    NT = min(512, L)          # matmul moving-dim tile
    LT = L // NT
    MT = M // NT
    SUB = NT // P
    MG = M // P               # m chunks for attention
    LSTRIP = min(1024, L)     # attention l-strip (PSUM budget)
    LS = L // LSTRIP
    SN = min(512, LSTRIP)
    LNS = LSTRIP // SN
    LN = LSTRIP // NT

    nc = bacc.Bacc()
    qT = nc.declare_dram_parameter("qT", [DIM, L], f32, isOutput=False)
    kT = nc.declare_dram_parameter("kT", [DIM, M], f32, isOutput=False)
    wqT = nc.declare_dram_parameter("wqT", [DIM, CW], f32, isOutput=False)
    wkT = nc.declare_dram_parameter("wkT", [DIM, CW], f32, isOutput=False)
    woT = nc.declare_dram_parameter("woT", [CW, DIM], f32, isOutput=False)
    out = nc.declare_dram_parameter("out", [L, DIM], f32, isOutput=True)
    den_dram = nc.dram_tensor("den_scratch", [HPC, L], f32)
    rden_dram = nc.dram_tensor("rden_scratch", [HPC, L], f32)

    with tile.TileContext(nc) as tc:
        with (
            tc.tile_pool(name="singles", bufs=1) as singles,
            tc.tile_pool(name="io", bufs=3) as io,
            tc.tile_pool(name="es", bufs=3) as es_pool,
            tc.tile_pool(name="opool", bufs=3) as opool,
        ):
            wq_sb = singles.tile([P, KC, CW], f32r)
            nc.sync.dma_start(wq_sb, wqT.rearrange("(kc p) c -> p kc c", p=P).bitcast(f32r))
            wk_sb = singles.tile([P, KC, CW], f32r)
            nc.sync.dma_start(wk_sb, wkT.rearrange("(kc p) c -> p kc c", p=P).bitcast(f32r))
            wo_sb = singles.tile([P, G, DIM], bf16)
            wo_stage = io.tile([P, G, DIM], f32, tag="wos")
            nc.sync.dma_start(wo_stage, woT.rearrange("(g p) j -> p g j", p=P))
            nc.vector.tensor_copy(wo_sb, wo_stage)

            qhT = singles.tile([P, G, L], bf16)
            khT = singles.tile([P, G, M], bf16)
            khp = singles.tile([P, MG, HPC, CH], bf16)
            xu = singles.tile([P, G, L], bf16)
            dstage = singles.tile([1, HPC, L], f32)
            rdbc = singles.tile([P, G, L], f32)

            ones_sb = singles.tile([P, 1], f32)
            nc.vector.memset(ones_sb, 1.0)
            for mg in range(MG):
                nc.vector.tensor_copy(khp[:, mg, :, HD:CH],
                                      ones_sb[:, None, :].to_broadcast([P, HPC, 1]))

            # ---- projections ----
            with tc.tile_pool(name="psP", bufs=2, space="PSUM") as psP:
                # QhT[c, l] = wqT.T @ qT
                for lt in range(LT):
                    qt_t = io.tile([P, KC, NT], f32r, tag="io")
                    nc.sync.dma_start(
                        qt_t, qT[:, ts(lt, NT)].rearrange("(kc p) l -> p kc l", p=P).bitcast(f32r))
                    for g in range(G):
                        ps = psP.tile([P, NT], f32, tag="ps")
                        for kc in range(KC):
                            nc.tensor.matmul(ps, lhsT=wq_sb[:, kc, ts(g, P)], rhs=qt_t[:, kc],
                                             start=(kc == 0), stop=(kc == KC - 1))
                        nc.vector.tensor_copy(qhT[:, g, ts(lt, NT)], ps)
                # KhT[c, m] and Kh[m, c] from the same k chunks
                for mt in range(MT):
                    kt_t = io.tile([P, KC, NT], f32r, tag="io")
                    nc.sync.dma_start(
                        kt_t, kT[:, ts(mt, NT)].rearrange("(kc p) m -> p kc m", p=P).bitcast(f32r))
                    for g in range(G):
                        ps = psP.tile([P, NT], f32, tag="ps")
                        for kc in range(KC):
                            nc.tensor.matmul(ps, lhsT=wk_sb[:, kc, ts(g, P)], rhs=kt_t[:, kc],
                                             start=(kc == 0), stop=(kc == KC - 1))
                        nc.vector.tensor_copy(khT[:, g, ts(mt, NT)], ps)
                    for sub in range(SUB):
                        mg = mt * SUB + sub
                        ps2 = psP.tile([P, CW], f32, tag="ps2")
                        for kc in range(KC):
                            nc.tensor.matmul(ps2, lhsT=kt_t[:, kc, ts(sub, P)], rhs=wk_sb[:, kc, :],
                                             start=(kc == 0), stop=(kc == KC - 1))
                        for h in range(HPC):
                            nc.vector.tensor_copy(khp[:, mg, h, 0:HD], ps2[:, ts(h, HD)])

            # ---- attention ----
            # Software-pipelined emission: S(mc+1) is enqueued on the PE
            # before X(mc) so the PE never head-of-line blocks on exp(mc)
            # (keeps TensorE dense -> HAM stays at full clock).
            with (
                tc.tile_pool(name="psS", bufs=3, space="PSUM") as psS,
                tc.tile_pool(name="psX", bufs=1, space="PSUM") as psX,
            ):
                for h in range(HPC):
                    g, hh = divmod(h, 2)
                    pb = hh * HD

                    for lsi in range(LS):
                        def emit_s(mc, lsi=lsi, g=g, pb=pb):
                            sps = psS.tile([P, LSTRIP], f32, tag="s")
                            for ln in range(LNS):
                                nc.tensor.matmul(
                                    sps[:, ts(ln, SN)],
                                    lhsT=khT[pb:pb + HD, g, ts(mc, P)],
                                    rhs=qhT[pb:pb + HD, g, ds(lsi * LSTRIP + ln * SN, SN)],
                                    start=True, stop=True)
                            return sps

                        xps = psX.tile([CH, LSTRIP], f32, tag="x")
                        sps_cur = emit_s(0)
                        for mc in range(MG):
                            sps_next = emit_s(mc + 1) if mc + 1 < MG else None
                            es = es_pool.tile([P, LSTRIP], bf16, tag="es")
                            nc.scalar.activation(es, sps_cur, Exp, scale=0.125)
                            for ln in range(LN):
                                nc.tensor.matmul(
                                    xps[:, ts(ln, NT)],
                                    lhsT=khp[:, mc, h, :],
                                    rhs=es[:, ts(ln, NT)],
                                    start=(mc == 0), stop=(mc == MG - 1))
                            sps_cur = sps_next
                        nc.vector.tensor_copy(xu[pb:pb + HD, g, ds(lsi * LSTRIP, LSTRIP)], xps[0:HD])
                        nc.vector.tensor_copy(dstage[0:1, h, ds(lsi * LSTRIP, LSTRIP)], xps[HD:CH])

            # ---- normalize ----
            # reciprocal on one partition is ~50us; bounce through DRAM to
            # spread the 4*L denominators over 128 partitions first.
            FSP = HPC * L // P
            dsp = singles.tile([P, FSP], f32)
            nc.sync.dma_start(den_dram[:, :].unsqueeze(0), dstage[0:1, :, :])
            nc.sync.dma_start(dsp, den_dram[:, :].rearrange("h (p f) -> (h p) f", p=P // HPC))
            nc.vector.reciprocal(dsp, dsp)
            nc.sync.dma_start(rden_dram[:, :].rearrange("h (p f) -> (h p) f", p=P // HPC), dsp)
            for h in range(HPC):
                g, hh = divmod(h, 2)
                nc.sync.dma_start(rdbc[ts(hh, HD), g, :], rden_dram[h:h + 1, :].to_broadcast([HD, L]))
            for g in range(G):
                nc.vector.tensor_mul(xu[:, g, :], xu[:, g, :], rdbc[:, g, :])

            # ---- output projection ----
            with tc.tile_pool(name="psO", bufs=4, space="PSUM") as psO:
                for lc in range(L // P):
                    for jt in range(JT):
                        po = psO.tile([P, 512], f32, tag="po")
                        for cc in range(G):
                            nc.tensor.matmul(po, lhsT=xu[:, cc, ts(lc, P)],
                                             rhs=wo_sb[:, cc, ts(jt, 512)],
                                             start=(cc == 0), stop=(cc == G - 1))
                        ot = opool.tile([P, 512], f32, tag="ot")
                        nc.vector.tensor_copy(ot, po)
                        nc.sync.dma_start(out[ts(lc, P), ts(jt, 512)], ot)

    nc.finalize()
    return nc


def _get_nc(L, M):
    key = (L, M)
    if key not in _cache:
        _cache[key] = _build(L, M)
    return _cache[key]


# head-major channel permutation: new channel c = h*64+d <- original column d*16+h
_PERM = np.array([(c % HD) * NH + c // HD for c in range(DIM)])

last_exec_time_ns = None
last_results = None


def kernel(q, k, v, Wq, Wk, Wv, Wo):  # noqa: ARG001 - v/Wv dead in reference
    global last_exec_time_ns, last_results
    q = np.asarray(q, np.float32)
    k = np.asarray(k, np.float32)
    Wq = np.asarray(Wq, np.float32)
    Wk = np.asarray(Wk, np.float32)
    Wo = np.asarray(Wo, np.float32)
    B, L, _ = q.shape
    M = k.shape[1]

    Wq_p = Wq[_PERM]            # (1024, 1024) head-major rows
    Wk_p = Wk[_PERM]
    WoT_p = Wo[:, _PERM].T      # (1024 c, 1024 j)

    qT = [np.ascontiguousarray(q[b].T) for b in range(B)]
    kT = [np.ascontiguousarray(k[b].T) for b in range(B)]
    wqT = [np.ascontiguousarray(Wq_p[hg * CW:(hg + 1) * CW, :].T) for hg in range(4)]
    wkT = [np.ascontiguousarray(Wk_p[hg * CW:(hg + 1) * CW, :].T) for hg in range(4)]
    woT = [np.ascontiguousarray(WoT_p[hg * CW:(hg + 1) * CW, :]) for hg in range(4)]

    in_maps = []
    for core in range(8):
        b, hg = divmod(core, 4)
        in_maps.append({"qT": qT[b], "kT": kT[b], "wqT": wqT[hg],
                        "wkT": wkT[hg], "woT": woT[hg]})

    nc = _get_nc(L, M)
    trace = bool(int(os.environ.get("MHA_TRACE", "0")))
    res = run_bass_kernel_spmd(nc, in_maps, core_ids=list(range(8)), trace=trace)
    last_results = res
    last_exec_time_ns = res.exec_time_ns

    out = np.zeros((B, L, DIM), np.float32)
    for core in range(8):
        b = core // 4
        out[b] += res.results[core]["out"]
    return out
